# revision 1
# baseline (speedup 1.0000x reference)
"""Trainium2 Bass kernel for ContextQueryAttn (BiDAF-style trilinear attention).

Computes, per batch b:
    sim = sc[:,None] + sq[None,:] + (ctx*wm) @ query.T          (Lc, Lq)
    sim = where(cmask[:,None] | qmask[None,:], -1e30, sim)
    S   = softmax(sim, axis=-1)   (row softmax over Lq)
    SS  = softmax(sim, axis=0)    (col softmax over Lc)
    A   = S @ query               (Lc, D)
    T   = SS.T @ ctx              (Lq, D)
    B   = S @ T                   (Lc, D)
returns (A, B).

v4 strategy (on top of bf16 + context-mask compaction):
 - the sim matrix is exponentiated ONCE: Pc = exp(cross) in [c', q]
   orientation.  Per-row factors e^{sc[c]} cancel in the row softmax and
   per-col factors e^{sq[q]} cancel in the column softmax, so
     * e^{sc} is folded into the ctx rows (host) for the T path, and
     * e^{sq_bias} is folded into the A/B rhs operands (host / csrec),
   making PT an EXACT transpose of Pc — done by DMA xbar transposes
   (zero compute-engine cost).  The qmask blend disappears (masked q
   rows scale to 0; host reconstructs them from ctxsum).
 - A|B numerators accumulate in one 2-bank psum tile, drained by one
   strided copy per ci alternating ACT/DVE; rowsum ships as a bf16 col.
 - outputs flushed in chunks; inputs split across DMA queues.
"""

import numpy as np
import ml_dtypes

import concourse.bass as bass
import concourse.tile as tile
from concourse import bacc, mybir
from concourse.bass_utils import run_bass_kernel_spmd

F32 = mybir.dt.float32
BF16 = mybir.dt.bfloat16
NPBF16 = ml_dtypes.bfloat16
EXP = mybir.ActivationFunctionType.Exp
ALU = mybir.AluOpType

B, LC, LQ, D = 32, 2048, 256, 256
NCORES = 8
BPC = B // NCORES          # batches per core
NKD = D // 128             # 2 contraction chunks over D
NQT = LQ // 128            # 2 query tiles
NEG = np.float32(-1e30)


def _build_kernel(tc, nc, ins, outs, NC2):
    import contextlib
    NT = NC2 // 128
    ctx = contextlib.ExitStack()

    sb = lambda name, bufs: ctx.enter_context(tc.tile_pool(name=name, bufs=bufs))
    psA = ctx.enter_context(tc.tile_pool(name="psA", bufs=2, space="PSUM"))
    psPc = ctx.enter_context(tc.tile_pool(name="psPc", bufs=2, space="PSUM"))
    psT = ctx.enter_context(tc.tile_pool(name="psT", bufs=2, space="PSUM"))

    p_ctxT = sb("pctxT", 2)
    p_ctx = sb("pctx", 2)
    p_qwm = sb("pqwm", 2)
    p_qe = sb("pqe", 2)
    p_fv = sb("pfv", 2)
    p_pt = sb("ppt", 2)
    p_pc = sb("ppc", 2)
    p_tn = sb("ptn", 2)
    p_cs = sb("pcs", 2)
    p_ast = sb("past", 2)

    ci_pairs = []
    i = 0
    while i < NT:
        ci_pairs.append((i, min(2, NT - i)))
        i += 2

    for b in range(BPC):
        # ---- loads (SBUF-image layouts; split for queue parallelism) ----
        qwm_sb = p_qwm.tile([128, NKD, LQ], BF16, name="qwm_sb")
        nc.sync.dma_start(out=qwm_sb[:], in_=ins["qwmT2"][b])
        fv_sb = p_fv.tile([128, NQT], F32, name="fv_sb")
        nc.sync.dma_start(out=fv_sb[:], in_=ins["fvec"][b])
        ctxT_sb = p_ctxT.tile([128, NKD, NC2], BF16, name="ctxT_sb")
        off = 0
        while off < NC2:
            cw = min(512, NC2 - off)
            for kd in range(NKD):
                nc.sync.dma_start(out=ctxT_sb[:, kd, off:off + cw],
                                  in_=ins["ctxT2"][b, :, kd, off:off + cw])
            off += cw
        ctx_sb = p_ctx.tile([128, NT, 258], BF16, name="ctx_sb")
        h = (NT + 1) // 2
        nc.sync.dma_start(out=ctx_sb[:, 0:h, :], in_=ins["ctx2"][b, :, 0:h])
        nc.sync.dma_start(out=ctx_sb[:, h:NT, :], in_=ins["ctx2"][b, :, h:NT])
        qe_sb = p_qe.tile([128, NQT, 257], BF16, name="qe_sb")
        nc.sync.dma_start(out=qe_sb[:], in_=ins["qe2"][b])
        sqb = lambda qt: fv_sb[:, qt:qt + 1]

        # ---- PT = exp(simT + sq_bias[q]) [q, c'] and Pc = exp(cross)
        #      [c', q] (e^{sc} folded into ctx rows), with T accumulation.
        #      PT groups, Pc pairs and T chains are interleaved so the
        #      ACT EXPs always have independent PE work to hide behind. ----
        PT_sb = p_pt.tile([128, NQT, NC2], BF16, name="PT_sb")
        Pc_sb = p_pc.tile([128, NT * LQ], BF16, name="Pc_sb")
        T_ps = [psT.tile([128, 512], F32, tag="psT", name=f"T_ps{qt}")
                for qt in range(NQT)]

        def emit_pt(qt, off, gw):
            ps = psA.tile([128, 1024], F32, tag="psA", name="ps_pt")
            o2 = 0
            while o2 < gw:
                cw = min(512, gw - o2)
                for kd in range(NKD):
                    nc.tensor.matmul(
                        ps[:, o2:o2 + cw],
                        lhsT=qwm_sb[:, kd, bass.ts(qt, 128)],
                        rhs=ctxT_sb[:, kd, off + o2:off + o2 + cw],
                        start=(kd == 0), stop=(kd == NKD - 1))
                o2 += cw
            nc.scalar.activation(
                PT_sb[:, qt, off:off + gw], ps[:, 0:gw], EXP, bias=sqb(qt))

        def emit_pc(pi):
            ci0, w = ci_pairs[pi]
            psc = psPc.tile([128, 512], F32, tag="psPc", name="psc")
            for j in range(w):
                for kd in range(NKD):
                    nc.tensor.matmul(
                        psc[:, j * LQ:(j + 1) * LQ],
                        lhsT=ctxT_sb[:, kd, bass.ts(ci0 + j, 128)],
                        rhs=qwm_sb[:, kd, :],
                        start=(kd == 0), stop=(kd == NKD - 1))
            nc.scalar.activation(
                Pc_sb[:, ci0 * LQ:(ci0 + w) * LQ], psc[:, 0:w * LQ], EXP)

        def emit_t(ci):
            for qt in range(NQT):
                nc.tensor.matmul(
                    T_ps[qt][:, 0:258],
                    lhsT=Pc_sb[:, ci * LQ + qt * 128:ci * LQ + qt * 128 + 128],
                    rhs=ctx_sb[:, ci, :],
                    start=(ci == 0), stop=(ci == NT - 1))

        pt_units = []
        for qt in range(NQT):
            off = 0
            while off < NC2:
                gw = min(1024, NC2 - off)
                pt_units.append((qt, off, gw))
                off += gw
        npc = len(ci_pairs)
        tq = []                 # T chains pending emission (need EXP done)
        for i in range(max(len(pt_units), npc + 1)):
            if i < len(pt_units):
                emit_pt(*pt_units[i])
            if i < npc:
                emit_pc(i)
            if i >= 1 and i - 1 < npc:
                ci0, w = ci_pairs[i - 1]
                tq.extend(range(ci0, ci0 + w))
            while len(tq) > 2:  # keep ~1 pair of lag behind the Pc EXPs
                emit_t(tq.pop(0))
        for ci in tq:
            emit_t(ci)

        # ---- T finalize: normalize (masked-q rows of Tn are garbage the
        #      host ignores; PT's zero rows annihilate them in B) ----
        Tn_sb = p_tn.tile([128, NQT, 256], BF16, name="Tn_sb")
        csrec = p_cs.tile([128, NQT], F32, name="csrec")
        for qt in range(NQT):
            nc.vector.reciprocal(csrec[:, qt:qt + 1], T_ps[qt][:, 256:257])
            nc.vector.tensor_scalar_mul(
                Tn_sb[:, qt, :], T_ps[qt][:, 0:256], csrec[:, qt:qt + 1])
        nc.sync.dma_start(out=outs["Tno"][b], in_=Tn_sb[:])

        # ---- A_num | B_num in one 2-bank psum tile; rowsum = bf16 col ----
        ABst = p_ast.tile([128, NT * 514], BF16, name="ABst")
        ab_flush = 0
        for ci in range(NT):
            pab = psA.tile([128, 1024], F32, tag="psA", name="pab")
            for qt in range(NQT):
                nc.tensor.matmul(
                    pab[:, 0:257],
                    lhsT=PT_sb[:, qt, bass.ts(ci, 128)],
                    rhs=qe_sb[:, qt, :],
                    start=(qt == 0), stop=(qt == NQT - 1))
            for qt in range(NQT):
                nc.tensor.matmul(
                    pab[:, 512:768],
                    lhsT=PT_sb[:, qt, bass.ts(ci, 128)],
                    rhs=Tn_sb[:, qt, :],
                    start=(qt == 0), stop=(qt == NQT - 1))
            # drain both halves in parallel: A+rowsum on ACT, B on DVE
            nc.scalar.copy(ABst[:, ci * 514:ci * 514 + 257], pab[:, 0:257])
            nc.vector.tensor_copy(
                ABst[:, ci * 514 + 257:(ci + 1) * 514], pab[:, 512:769])
            if ci >= 1 and (ci % 2 == 1 or ci == NT - 1):
                nc.sync.dma_start(
                    out=outs["ABo"][b, :, ab_flush * 514:(ci + 1) * 514],
                    in_=ABst[:, ab_flush * 514:(ci + 1) * 514])
                ab_flush = ci + 1

    ctx.close()


def build_program(NC2):
    NT = NC2 // 128
    nc = bacc.Bacc("TRN2", target_bir_lowering=False, debug=False,
                   num_devices=NCORES)
    ins = {
        "ctxT2": nc.dram_tensor("ctxT2", [BPC, 128, NKD, NC2], BF16,
                                kind="ExternalInput").ap(),
        "ctx2": nc.dram_tensor("ctx2", [BPC, 128, NT, 258], BF16,
                               kind="ExternalInput").ap(),
        "qwmT2": nc.dram_tensor("qwmT2", [BPC, 128, NKD, LQ], BF16,
                                kind="ExternalInput").ap(),
        "qe2": nc.dram_tensor("qe2", [BPC, 128, NQT, 257], BF16,
                              kind="ExternalInput").ap(),
        "fvec": nc.dram_tensor("fvec", [BPC, 128, NQT], F32,
                               kind="ExternalInput").ap(),
    }
    outs = {
        "ABo": nc.dram_tensor("ABo", [BPC, 128, NT * 514], BF16,
                              kind="ExternalOutput").ap(),
        "Tno": nc.dram_tensor("Tno", [BPC, 128, NQT, 256], BF16,
                              kind="ExternalOutput").ap(),
    }
    with tile.TileContext(nc) as tc:
        _build_kernel(tc, nc, ins, outs, NC2)
    nc.compile()
    return nc


def _aux(context_mask):
    """Per-batch unmasked-context indices and the padded compact size."""
    cm = np.asarray(context_mask).astype(bool)
    idx = [np.flatnonzero(~cm[b]) for b in range(cm.shape[0])]
    nmax = max((len(u) for u in idx), default=1)
    NC2 = max(256, ((int(nmax) + 127) // 128) * 128)
    return idx, NC2


def _img(a, p=128):
    """[N*p, X...] row-major -> SBUF image [p, N, X...] (row r = t*p + lane)."""
    n = a.shape[0] // p
    return np.ascontiguousarray(
        a.reshape((n, p) + a.shape[1:]).swapaxes(0, 1))


def host_prep(context, query, context_mask, query_mask, w0):
    """Host-side preprocessing: compact, shard, build device blobs."""
    f = np.float32
    context = np.asarray(context, dtype=f)
    query = np.asarray(query, dtype=f)
    w0 = np.asarray(w0, dtype=f)
    wc, wq, wm = w0[:D], w0[D:2 * D], w0[2 * D:]
    qmf = np.asarray(query_mask).astype(f)                  # (B, LQ)
    idx, NC2 = _aux(context_mask)
    NT = NC2 // 128

    sq = query @ wq                                         # (B, LQ)
    sq_bias = ((1.0 - qmf) * sq + qmf * NEG).astype(f)      # -1e30 on masked q
    qwmT = (query * wm).transpose(0, 2, 1)                  # (B, D, LQ) f32
    qe = np.concatenate([query, np.ones((B, LQ, 1), f)], -1)  # (B, LQ, 257)

    in_maps = []
    for c in range(NCORES):
        m = {"ctxT2": np.zeros((BPC, 128, NKD, NC2), NPBF16),
             "ctx2": np.zeros((BPC, 128, NT, 258), NPBF16),
             "qwmT2": np.empty((BPC, 128, NKD, LQ), NPBF16),
             "qe2": np.empty((BPC, 128, NQT, 257), NPBF16),
             "fvec": np.zeros((BPC, 128, NQT), f)}
        for lb in range(BPC):
            b = c * BPC + lb
            U = idx[b]
            n = len(U)
            cU = context[b][U]                              # (n, D)
            scU = cU @ wc                                   # (n,)
            ctxT_pad = np.zeros((D, NC2), f)
            ctxT_pad[:, :n] = cU.T
            m["ctxT2"][lb] = _img(ctxT_pad).astype(NPBF16)
            # ctx rows scaled by e^{sc[c]} (column-softmax weight); the
            # ones-col picks up the same factor => correct normalizer.
            ctx_pad = np.zeros((NC2, 258), f)
            ctx_pad[:n, :D] = cU
            ctx_pad[:n, D] = 1.0
            ctx_pad[:n] *= np.exp(scU, dtype=f)[:, None]
            m["ctx2"][lb] = _img(ctx_pad).astype(NPBF16)
            m["qwmT2"][lb] = _img(qwmT[b]).astype(NPBF16)
            m["qe2"][lb] = _img(qe[b]).astype(NPBF16)
            m["fvec"][lb] = sq_bias[b].reshape(NQT, 128).T
        in_maps.append(m)
    return in_maps


_cached_nc = {}


def get_program(NC2):
    if NC2 not in _cached_nc:
        _cached_nc[NC2] = build_program(NC2)
    return _cached_nc[NC2]


def run_on_hw(in_maps, **kwargs):
    NC2 = in_maps[0]["ctxT2"].shape[-1]
    nc = get_program(NC2)
    return run_bass_kernel_spmd(nc, in_maps, core_ids=list(range(NCORES)),
                                **kwargs)


def kernel(context, query, context_mask, query_mask, w0):
    f = np.float32
    context = np.asarray(context, dtype=f)
    query = np.asarray(query, dtype=f)
    w0 = np.asarray(w0, dtype=f)
    qmask = np.asarray(query_mask).astype(bool)
    idx, NC2 = _aux(context_mask)
    NT = NC2 // 128
    ctxmean = context.mean(1, dtype=np.float64).astype(f)   # (B, D)
    in_maps = host_prep(context, query, context_mask, query_mask, w0)
    res = run_on_hw(in_maps)

    A = np.empty((B, LC, D), f)
    Bm = np.empty((B, LC, D), f)
    cmask = np.asarray(context_mask).astype(bool)
    for c in range(NCORES):
        r = res.results[c]
        for lb in range(BPC):
            b = c * BPC + lb
            U = idx[b]
            n = len(U)
            ABr = r["ABo"][lb].astype(f).reshape(128, NT, 514).swapaxes(0, 1)
            ABr = ABr.reshape(NC2, 514)
            Tn = r["Tno"][lb].astype(f).swapaxes(0, 1).reshape(LQ, D)
            # masked-q rows of the device Tn are garbage; true value = ctxmean
            Tn_true = np.where(qmask[b][:, None], ctxmean[b][None, :], Tn)
            inv = 1.0 / ABr[:n, 256:257]
            A[b][U] = ABr[:n, 0:256] * inv
            Bm[b][U] = ABr[:n, 257:513] * inv
            mrow = cmask[b]
            A[b][mrow] = query[b].mean(0, dtype=np.float64).astype(f)
            Bm[b][mrow] = Tn_true.mean(0, dtype=np.float64).astype(f)
    return A, Bm

